# revision 10
# baseline (speedup 1.0000x reference)
"""Trainium2 Bass kernel for nn_ContinuousEmbedding (masked matmul + bias).

Computes out = x @ (weights * mask) + bias, reshaped to [B, in_size, out_size],
where mask zeroes each input feature's own [out_size]-wide diagonal block.

Strategy: tensor-parallel across the 8 NeuronCores by splitting the
in_size*out_size (=16384) output columns into 8 shards of 2048 columns.
The mask is constant and folded into the weights on the host.

All-bf16, transposed output: everything is cast to bf16 on the host
(tolerance is 2e-2 rel-l2; bf16 end-to-end costs ~3e-3), halving both the
input loads (3 MB/core) and the output stores (16 MB/core) so the DMA time
(~19 MB / ~330 GB/s) sits right at the PE floor (256 matmuls x 512 rows
@ 2.4 GHz = 54.6 us).  The matmul is transposed: the stationary operand is
a W column-tile, the moving operand is x^T, so PSUM tiles hold
[128 out-cols, batch].  With output COLUMNS on the partition axis the bias
is a per-partition scalar, so both PSUM-capable element-wise engines (DVE
tensor_scalar_add, Act activation-Identity) evict PSUM->SBUF with the bias
add and fp32->bf16 cast fused.

v5 refinements (from NTFF traces):
 - Batch chunks taper: [512, 1024, 1024, 1024, 256, 256].  The 1024-pairs
   amortize LDWEIGHTS; the small final chunks mean the last evictions (and
   so the last stores) involve little data, shrinking the end-of-kernel
   DMA tail (the stores of a chunk can only start once it is computed).
 - PSUM tiles are fixed [128, 1024] (2 banks, 4 bufs, bank-aligned); small
   chunks use a prefix slice.  Evictions alternate DVE/Act.
 - Stores are split into ~6 pieces per chunk, rotating across the three
   DMA rings (SP + Act HWDGE, Pool SWDGE) so each ring's transfers start
   as soon as its tiles evict and the wire stays evenly fed.
 - First x^T / W pieces are k-split (128KB) so the first matmul's
   dependencies land as early as the framework prologue allows.
 - The TileContext epilogue serially resets the ENTIRE kernel semaphore
   range (~250 sems x ~115 ns spread over 5 engines ~= 6 us); we shrink
   the range via get_kernel_semaphore_range so only what the scheduler
   needs gets reset.
"""

import numpy as np
import ml_dtypes

B = 4096
IN_SIZE = 256
OUT_SIZE = 64
IO = IN_SIZE * OUT_SIZE          # 16384
N_CORES = 8
N_SHARD = IO // N_CORES          # 2048 output columns per core
P = 128                          # SBUF partitions
KO = IN_SIZE // P                # 2 contraction sub-tiles
M_TILE = 512                     # matmul moving free dim (= PSUM bank, fp32)
M_PAIR = 2 * M_TILE              # 1024: two matmuls per stationary load
M_PAIRS = B // M_PAIR            # 4 batch pair-chunks
NT = N_SHARD // P                # 16 column tiles (out partitions) per core

BF16 = np.dtype(ml_dtypes.bfloat16)

_CACHE: dict = {}


SEM_BUDGET = 120                 # shrunk kernel semaphore range (epilogue
                                 # resets scale with the range size)


def _build_program(sem_budget=None):
    import concourse.mybir as mybir
    import concourse.tile as tile
    import concourse.bass as bass_mod
    from concourse import bacc

    sem_budget = sem_budget or SEM_BUDGET
    orig_fn = bass_mod.get_kernel_semaphore_range
    orig = orig_fn()
    if sem_budget and orig.start + sem_budget < orig.stop:
        bass_mod.get_kernel_semaphore_range = lambda: range(
            orig.start, orig.start + sem_budget
        )
    try:
        nc = bacc.Bacc(
            "TRN2", target_bir_lowering=False, debug=False,
            num_devices=N_CORES,
        )
    finally:
        bass_mod.get_kernel_semaphore_range = orig_fn
    bf = mybir.dt.bfloat16
    f32 = mybir.dt.float32
    xt = nc.dram_tensor("xt", [KO, P, B], bf, kind="ExternalInput").ap()
    w = nc.dram_tensor("w", [KO, P, N_SHARD], bf, kind="ExternalInput").ap()
    # bias pre-transposed on host to [P, NT] (partition-major).
    bias = nc.dram_tensor("bias", [P, NT], f32, kind="ExternalInput").ap()
    # out^T: [n_cols, batch]; host transposes back.
    out = nc.dram_tensor("out", [N_SHARD, B], bf, kind="ExternalOutput").ap()

    with tile.TileContext(nc) as tc:
        with tc.tile_pool(name="const", bufs=1) as const, \
             tc.tile_pool(name="psum", bufs=4, space="PSUM") as psum_pool, \
             tc.tile_pool(name="stage", bufs=2) as stage_pool:
            xt_sb = const.tile([P, KO, B], bf)
            w_sb = const.tile([P, KO, N_SHARD], bf)
            bias_sb = const.tile([P, NT], f32)

            # Loads.  SP ring: W column-chunks (k-split first chunk so the
            # first LDWEIGHTS fires ASAP); Act ring: k-split first x^T
            # piece, bias, then the rest of x^T.  Per-partition runs are
            # 1-4KB so HWDGE descriptors stay fat.
            w_src = w.rearrange("k p n -> p k n")
            xt_src = xt.rearrange("k p m -> p k m")
            wc = N_SHARD // 4
            nc.sync.dma_start(
                out=w_sb[:, 0:1, 0:wc], in_=w_src[:, 0:1, 0:wc]
            )
            nc.scalar.dma_start(
                out=xt_sb[:, 0:1, 0:512], in_=xt_src[:, 0:1, 0:512]
            )
            nc.sync.dma_start(
                out=w_sb[:, 1:2, 0:wc], in_=w_src[:, 1:2, 0:wc]
            )
            nc.scalar.dma_start(
                out=xt_sb[:, 1:2, 0:512], in_=xt_src[:, 1:2, 0:512]
            )
            nc.scalar.dma_start(out=bias_sb[:], in_=bias[:])
            for i in range(1, 4):
                cs = slice(i * wc, (i + 1) * wc)
                nc.sync.dma_start(out=w_sb[:, :, cs], in_=w_src[:, :, cs])
            for lo, hi in [(512, 1024), (1024, 2560), (2560, B)]:
                ms = slice(lo, hi)
                nc.scalar.dma_start(out=xt_sb[:, :, ms], in_=xt_src[:, :, ms])

            # Batch chunks taper so the last stores are small.
            CHUNKS = [(0, 512), (512, 1536), (1536, 2560), (2560, 3584),
                      (3584, 3840), (3840, 4096)]
            # Store piece boundaries (in t-tiles) per chunk; rings rotate.
            PIECES = [0, 3, 6, 9, 12, 14, NT]
            RINGS = [nc.sync, nc.scalar, nc.gpsimd]

            out_r = out.rearrange("(t p) m -> p t m", p=P)
            for m, (mlo, mhi) in enumerate(CHUNKS):
                csz = mhi - mlo
                ms = slice(mlo, mhi)
                # Fixed shape so the pool ring-rotates 2 slots; small
                # chunks use a prefix slice.
                stage_full = stage_pool.tile([P, NT, M_PAIR], bf)
                stage = stage_full[:, :, 0:csz]
                for t in range(NT):
                    ns = slice(t * P, (t + 1) * P)
                    # Fixed 2-bank tile keeps PSUM bank alignment; small
                    # chunks use a prefix slice.
                    ps = psum_pool.tile([P, M_PAIR], f32)
                    for k in range(KO):
                        st, sp = (k == 0), (k == KO - 1)
                        # One LDWEIGHTS per (k, t) covers every 512-wide
                        # moving slice of this chunk.
                        for s0 in range(0, csz, M_TILE):
                            s1 = min(s0 + M_TILE, csz)
                            nc.tensor.matmul(
                                ps[:, s0:s1], lhsT=w_sb[:, k, ns],
                                rhs=xt_sb[:, k, mlo + s0:mlo + s1],
                                start=st, stop=sp,
                            )
                    # PSUM->SBUF eviction with fused bias add + bf16 cast,
                    # alternating DVE / Act (Pool cannot read PSUM on TRN2).
                    dst = stage[:, t, :]
                    bs = bias_sb[:, t:t + 1]
                    if t % 2 == 0:
                        nc.vector.tensor_scalar_add(dst, ps[:, 0:csz], bs)
                    else:
                        nc.scalar.activation(
                            dst, ps[:, 0:csz],
                            mybir.ActivationFunctionType.Identity,
                            bias=bs, scale=1.0,
                        )
                # Stores: ~6 pieces per chunk rotated over the three rings
                # (SP + Act HWDGE, Pool SWDGE) -- each fires as soon as its
                # tiles evict, keeping the wire evenly fed.
                for i in range(len(PIECES) - 1):
                    lo, hi = PIECES[i], PIECES[i + 1]
                    eng = RINGS[(m + i) % 3]
                    eng.dma_start(
                        out=out_r[:, lo:hi, ms], in_=stage[:, lo:hi, :]
                    )

    nc.compile()
    return nc


def _get_program(mode=None):
    if "prog" not in _CACHE:
        _CACHE["prog"] = _build_program()
    return _CACHE["prog"]


def _shard_inputs(x, weights, bias, mode=None):
    # Fold the constant block-diagonal mask into the weights on the host.
    col_block = np.arange(IO, dtype=np.int64) // OUT_SIZE
    mask = (col_block[None, :] != np.arange(IN_SIZE)[:, None])
    wm = weights * mask.astype(weights.dtype)
    xt16 = x.T.astype(BF16).reshape(KO, P, B)
    in_maps = []
    for c in range(N_CORES):
        sl = slice(c * N_SHARD, (c + 1) * N_SHARD)
        w16 = wm[:, sl].astype(BF16).reshape(KO, P, N_SHARD)
        bias_t = np.ascontiguousarray(
            bias[sl].astype(np.float32).reshape(NT, P).T
        )
        in_maps.append({
            "xt": xt16,
            "w": np.ascontiguousarray(w16),
            "bias": bias_t,
        })
    return in_maps


def run_sharded(in_maps, mode=None, **kwargs):
    """Run the SPMD program on cores 0-7. kwargs forwarded (e.g. trace)."""
    from concourse.bass_utils import run_bass_kernel_spmd

    nc = _get_program()
    return run_bass_kernel_spmd(
        nc, in_maps, core_ids=list(range(N_CORES)), **kwargs
    )


def kernel(x: np.ndarray, weights: np.ndarray, bias: np.ndarray) -> np.ndarray:
    x = np.asarray(x, dtype=np.float32)
    weights = np.asarray(weights, dtype=np.float32)
    bias = np.asarray(bias, dtype=np.float32)
    in_maps = _shard_inputs(x, weights, bias)
    res = run_sharded(in_maps)
    # Each core returns out^T [N_SHARD, B] bf16; transpose back and upcast.
    full = np.concatenate(
        [np.asarray(res.results[c]["out"]).T for c in range(N_CORES)], axis=1
    ).astype(np.float32)
    return full.reshape(B, IN_SIZE, OUT_SIZE)


# revision 13
# speedup vs baseline: 1.0607x; 1.0607x over previous
"""Trainium2 Bass kernel for nn_ContinuousEmbedding (masked matmul + bias).

Computes out = x @ (weights * mask) + bias, reshaped to [B, in_size, out_size],
where mask zeroes each input feature's own [out_size]-wide diagonal block.

Strategy: tensor-parallel across the 8 NeuronCores by splitting the
in_size*out_size (=16384) output columns into 8 shards of 2048 columns.
The mask is constant and folded into the weights on the host.

All-bf16, transposed output: everything is cast to bf16 on the host
(tolerance is 2e-2 rel-l2; bf16 end-to-end costs ~3e-3), halving both the
input loads (3 MB/core) and the output stores (16 MB/core) so the DMA time
(~19 MB / ~330 GB/s) sits right at the PE floor (256 matmuls x 512 rows
@ 2.4 GHz = 54.6 us).  The matmul is transposed: the stationary operand is
a W column-tile, the moving operand is x^T, so PSUM tiles hold
[128 out-cols, batch].  With output COLUMNS on the partition axis the bias
is a per-partition scalar, so both PSUM-capable element-wise engines (DVE
tensor_scalar_add, Act activation-Identity) evict PSUM->SBUF with the bias
add and fp32->bf16 cast fused.

v5 refinements (from NTFF traces):
 - Batch chunks taper: [512, 1024, 1024, 1024, 256, 256].  The 1024-pairs
   amortize LDWEIGHTS; the small final chunks mean the last evictions (and
   so the last stores) involve little data, shrinking the end-of-kernel
   DMA tail (the stores of a chunk can only start once it is computed).
 - PSUM tiles are fixed [128, 1024] (2 banks, 4 bufs, bank-aligned); small
   chunks use a prefix slice.  Evictions alternate DVE/Act.
 - Stores are split into ~6 pieces per chunk, rotating across the three
   DMA rings (SP + Act HWDGE, Pool SWDGE) so each ring's transfers start
   as soon as its tiles evict and the wire stays evenly fed.
 - First x^T / W pieces are k-split (128KB) so the first matmul's
   dependencies land as early as the framework prologue allows.
 - partition_id machinery disabled (no collectives): removes per-engine
   register loads from the fixed prologue.  The remaining ~7 us prologue
   (engine rendezvous barriers) and ~6 us epilogue (256 serial semaphore
   resets, compiler-managed) are framework-fixed.
"""

import numpy as np
import ml_dtypes

B = 4096
IN_SIZE = 256
OUT_SIZE = 64
IO = IN_SIZE * OUT_SIZE          # 16384
N_CORES = 8
N_SHARD = IO // N_CORES          # 2048 output columns per core
P = 128                          # SBUF partitions
KO = IN_SIZE // P                # 2 contraction sub-tiles
M_TILE = 512                     # matmul moving free dim (= PSUM bank, fp32)
M_PAIR = 2 * M_TILE              # 1024: two matmuls per stationary load
M_PAIRS = B // M_PAIR            # 4 batch pair-chunks
NT = N_SHARD // P                # 16 column tiles (out partitions) per core

BF16 = np.dtype(ml_dtypes.bfloat16)

_CACHE: dict = {}


def _build_program():
    import concourse.mybir as mybir
    import concourse.tile as tile
    from concourse import bacc

    # No collectives and no partition-dependent control flow: skip the
    # partition_id machinery (its per-engine register loads sit on the
    # critical prologue path).
    nc = bacc.Bacc(
        "TRN2", target_bir_lowering=False, debug=False,
        num_devices=N_CORES, enable_partition_id=False,
    )
    bf = mybir.dt.bfloat16
    f32 = mybir.dt.float32
    xt = nc.dram_tensor("xt", [KO, P, B], bf, kind="ExternalInput").ap()
    w = nc.dram_tensor("w", [KO, P, N_SHARD], bf, kind="ExternalInput").ap()
    # bias pre-transposed on host to [P, NT] (partition-major).
    bias = nc.dram_tensor("bias", [P, NT], f32, kind="ExternalInput").ap()
    # out^T: [n_cols, batch]; host transposes back.
    out = nc.dram_tensor("out", [N_SHARD, B], bf, kind="ExternalOutput").ap()

    with tile.TileContext(nc) as tc:
        with tc.tile_pool(name="const", bufs=1) as const, \
             tc.tile_pool(name="psum", bufs=4, space="PSUM") as psum_pool, \
             tc.tile_pool(name="stage", bufs=2) as stage_pool:
            xt_sb = const.tile([P, KO, B], bf)
            w_sb = const.tile([P, KO, N_SHARD], bf)
            bias_sb = const.tile([P, NT], f32)

            # Loads.  SP ring: W column-chunks (k-split first chunk so the
            # first LDWEIGHTS fires ASAP); Act ring: k-split first x^T
            # piece, bias, then the rest of x^T.  Per-partition runs are
            # 1-4KB so HWDGE descriptors stay fat.
            w_src = w.rearrange("k p n -> p k n")
            xt_src = xt.rearrange("k p m -> p k m")
            wc = N_SHARD // 4
            nc.sync.dma_start(
                out=w_sb[:, 0:1, 0:wc], in_=w_src[:, 0:1, 0:wc]
            )
            nc.scalar.dma_start(
                out=xt_sb[:, 0:1, 0:512], in_=xt_src[:, 0:1, 0:512]
            )
            nc.sync.dma_start(
                out=w_sb[:, 1:2, 0:wc], in_=w_src[:, 1:2, 0:wc]
            )
            nc.scalar.dma_start(
                out=xt_sb[:, 1:2, 0:512], in_=xt_src[:, 1:2, 0:512]
            )
            nc.scalar.dma_start(out=bias_sb[:], in_=bias[:])
            for i in range(1, 4):
                cs = slice(i * wc, (i + 1) * wc)
                nc.sync.dma_start(out=w_sb[:, :, cs], in_=w_src[:, :, cs])
            for lo, hi in [(512, 1024), (1024, 2560), (2560, B)]:
                ms = slice(lo, hi)
                nc.scalar.dma_start(out=xt_sb[:, :, ms], in_=xt_src[:, :, ms])

            # Batch chunks: 512-wide first (aligned with the k-split first
            # loads) and last (small final-store tail); 1024-pairs between.
            # Total matmul count stays at the floor of 256 (512 rows each).
            CHUNKS = [(0, 512), (512, 1536), (1536, 2560), (2560, 3584),
                      (3584, 4096)]
            # Store piece boundaries (in t-tiles) per chunk; rings rotate.
            PIECES = [0, 3, 6, 9, 12, 14, NT]
            RINGS = [nc.sync, nc.scalar, nc.gpsimd]

            out_r = out.rearrange("(t p) m -> p t m", p=P)
            for m, (mlo, mhi) in enumerate(CHUNKS):
                csz = mhi - mlo
                ms = slice(mlo, mhi)
                # Fixed shape so the pool ring-rotates 2 slots; small
                # chunks use a prefix slice.
                stage_full = stage_pool.tile([P, NT, M_PAIR], bf)
                stage = stage_full[:, :, 0:csz]
                for t in range(NT):
                    ns = slice(t * P, (t + 1) * P)
                    # Fixed 2-bank tile keeps PSUM bank alignment; small
                    # chunks use a prefix slice.
                    ps = psum_pool.tile([P, M_PAIR], f32)
                    for k in range(KO):
                        st, sp = (k == 0), (k == KO - 1)
                        # One LDWEIGHTS per (k, t) covers every 512-wide
                        # moving slice of this chunk.
                        for s0 in range(0, csz, M_TILE):
                            s1 = min(s0 + M_TILE, csz)
                            nc.tensor.matmul(
                                ps[:, s0:s1], lhsT=w_sb[:, k, ns],
                                rhs=xt_sb[:, k, mlo + s0:mlo + s1],
                                start=st, stop=sp,
                            )
                    # PSUM->SBUF eviction with fused bias add + bf16 cast,
                    # alternating DVE / Act (Pool cannot read PSUM on TRN2).
                    dst = stage[:, t, :]
                    bs = bias_sb[:, t:t + 1]
                    if t % 2 == 0:
                        nc.vector.tensor_scalar_add(dst, ps[:, 0:csz], bs)
                    else:
                        nc.scalar.activation(
                            dst, ps[:, 0:csz],
                            mybir.ActivationFunctionType.Identity,
                            bias=bs, scale=1.0,
                        )
                # Stores: ~6 pieces per chunk rotated over the three rings
                # (SP + Act HWDGE, Pool SWDGE) -- each fires as soon as its
                # tiles evict, keeping the wire evenly fed.
                for i in range(len(PIECES) - 1):
                    lo, hi = PIECES[i], PIECES[i + 1]
                    eng = RINGS[(m + i) % 3]
                    eng.dma_start(
                        out=out_r[:, lo:hi, ms], in_=stage[:, lo:hi, :]
                    )

    nc.compile()
    return nc


def _get_program(mode=None):
    if "prog" not in _CACHE:
        _CACHE["prog"] = _build_program()
    return _CACHE["prog"]


def _shard_inputs(x, weights, bias, mode=None):
    # Fold the constant block-diagonal mask into the weights on the host.
    col_block = np.arange(IO, dtype=np.int64) // OUT_SIZE
    mask = (col_block[None, :] != np.arange(IN_SIZE)[:, None])
    wm = weights * mask.astype(weights.dtype)
    xt16 = x.T.astype(BF16).reshape(KO, P, B)
    in_maps = []
    for c in range(N_CORES):
        sl = slice(c * N_SHARD, (c + 1) * N_SHARD)
        w16 = wm[:, sl].astype(BF16).reshape(KO, P, N_SHARD)
        bias_t = np.ascontiguousarray(
            bias[sl].astype(np.float32).reshape(NT, P).T
        )
        in_maps.append({
            "xt": xt16,
            "w": np.ascontiguousarray(w16),
            "bias": bias_t,
        })
    return in_maps


def run_sharded(in_maps, mode=None, **kwargs):
    """Run the SPMD program on cores 0-7. kwargs forwarded (e.g. trace)."""
    from concourse.bass_utils import run_bass_kernel_spmd

    nc = _get_program()
    return run_bass_kernel_spmd(
        nc, in_maps, core_ids=list(range(N_CORES)), **kwargs
    )


def kernel(x: np.ndarray, weights: np.ndarray, bias: np.ndarray) -> np.ndarray:
    x = np.asarray(x, dtype=np.float32)
    weights = np.asarray(weights, dtype=np.float32)
    bias = np.asarray(bias, dtype=np.float32)
    in_maps = _shard_inputs(x, weights, bias)
    res = run_sharded(in_maps)
    # Each core returns out^T [N_SHARD, B] bf16; transpose back and upcast.
    full = np.concatenate(
        [np.asarray(res.results[c]["out"]).T for c in range(N_CORES)], axis=1
    ).astype(np.float32)
    return full.reshape(B, IN_SIZE, OUT_SIZE)


# revision 15
# speedup vs baseline: 1.0692x; 1.0080x over previous
"""Trainium2 Bass kernel for nn_ContinuousEmbedding (masked matmul + bias).

Computes out = x @ (weights * mask) + bias, reshaped to [B, in_size, out_size],
where mask zeroes each input feature's own [out_size]-wide diagonal block.

Strategy: tensor-parallel across the 8 NeuronCores by splitting the
in_size*out_size (=16384) output columns into 8 shards of 2048 columns.
The mask is constant and folded into the weights on the host.

All-bf16, transposed output: everything is cast to bf16 on the host
(tolerance is 2e-2 rel-l2; bf16 end-to-end costs ~3e-3), halving both the
input loads (3 MB/core) and the output stores (16 MB/core) so the DMA time
(~19 MB / ~330 GB/s) sits right at the PE floor (256 matmuls x 512 rows
@ 2.4 GHz = 54.6 us).  The matmul is transposed: the stationary operand is
a W column-tile, the moving operand is x^T, so PSUM tiles hold
[128 out-cols, batch].  With output COLUMNS on the partition axis the bias
is a per-partition scalar, so both PSUM-capable element-wise engines (DVE
tensor_scalar_add, Act activation-Identity) evict PSUM->SBUF with the bias
add and fp32->bf16 cast fused.

v5 refinements (from NTFF traces):
 - Batch chunks taper: [512, 1024, 1024, 1024, 256, 256].  The 1024-pairs
   amortize LDWEIGHTS; the small final chunks mean the last evictions (and
   so the last stores) involve little data, shrinking the end-of-kernel
   DMA tail (the stores of a chunk can only start once it is computed).
 - PSUM tiles are fixed [128, 1024] (2 banks, 4 bufs, bank-aligned); small
   chunks use a prefix slice.  Evictions alternate DVE/Act.
 - Stores are split into ~6 pieces per chunk, rotating across the three
   DMA rings (SP + Act HWDGE, Pool SWDGE) so each ring's transfers start
   as soon as its tiles evict and the wire stays evenly fed.
 - First x^T / W pieces are k-split (128KB) so the first matmul's
   dependencies land as early as the framework prologue allows.
 - partition_id machinery disabled (no collectives): removes per-engine
   register loads from the fixed prologue.  The remaining ~7 us prologue
   (engine rendezvous barriers) and ~6 us epilogue (256 serial semaphore
   resets, compiler-managed) are framework-fixed.
"""

import numpy as np
import ml_dtypes

B = 4096
IN_SIZE = 256
OUT_SIZE = 64
IO = IN_SIZE * OUT_SIZE          # 16384
N_CORES = 8
N_SHARD = IO // N_CORES          # 2048 output columns per core
P = 128                          # SBUF partitions
KO = IN_SIZE // P                # 2 contraction sub-tiles
M_TILE = 512                     # matmul moving free dim (= PSUM bank, fp32)
M_PAIR = 2 * M_TILE              # 1024: two matmuls per stationary load
M_PAIRS = B // M_PAIR            # 4 batch pair-chunks
NT = N_SHARD // P                # 16 column tiles (out partitions) per core

BF16 = np.dtype(ml_dtypes.bfloat16)

_CACHE: dict = {}


def _build_program():
    import concourse.mybir as mybir
    import concourse.tile as tile
    from concourse import bacc

    # No collectives and no partition-dependent control flow: skip the
    # partition_id machinery (its per-engine register loads sit on the
    # critical prologue path).
    nc = bacc.Bacc(
        "TRN2", target_bir_lowering=False, debug=False,
        num_devices=N_CORES, enable_partition_id=False,
    )
    bf = mybir.dt.bfloat16
    f32 = mybir.dt.float32
    xt = nc.dram_tensor("xt", [KO, P, B], bf, kind="ExternalInput").ap()
    w = nc.dram_tensor("w", [KO, P, N_SHARD], bf, kind="ExternalInput").ap()
    # bias pre-transposed on host to [P, NT] (partition-major).
    bias = nc.dram_tensor("bias", [P, NT], f32, kind="ExternalInput").ap()
    # out^T: [n_cols, batch]; host transposes back.
    out = nc.dram_tensor("out", [N_SHARD, B], bf, kind="ExternalOutput").ap()

    with tile.TileContext(nc) as tc:
        with tc.tile_pool(name="const", bufs=1) as const, \
             tc.tile_pool(name="psum", bufs=4, space="PSUM") as psum_pool, \
             tc.tile_pool(name="stage", bufs=2) as stage_pool:
            xt_sb = const.tile([P, KO, B], bf)
            w_sb = const.tile([P, KO, N_SHARD], bf)
            bias_sb = const.tile([P, NT], f32)

            # Loads.  SP ring: W column-chunks (k-split first chunk so the
            # first LDWEIGHTS fires ASAP); Act ring: k-split first x^T
            # piece, bias, then the rest of x^T.  Per-partition runs are
            # 1-4KB so HWDGE descriptors stay fat.
            w_src = w.rearrange("k p n -> p k n")
            xt_src = xt.rearrange("k p m -> p k m")
            wc = N_SHARD // 4
            nc.sync.dma_start(
                out=w_sb[:, 0:1, 0:wc], in_=w_src[:, 0:1, 0:wc]
            )
            nc.scalar.dma_start(
                out=xt_sb[:, 0:1, 0:512], in_=xt_src[:, 0:1, 0:512]
            )
            nc.sync.dma_start(
                out=w_sb[:, 1:2, 0:wc], in_=w_src[:, 1:2, 0:wc]
            )
            nc.scalar.dma_start(
                out=xt_sb[:, 1:2, 0:512], in_=xt_src[:, 1:2, 0:512]
            )
            nc.scalar.dma_start(out=bias_sb[:], in_=bias[:])
            for i in range(1, 4):
                cs = slice(i * wc, (i + 1) * wc)
                nc.sync.dma_start(out=w_sb[:, :, cs], in_=w_src[:, :, cs])
            for lo, hi in [(512, 1024), (1024, 2560)]:
                ms = slice(lo, hi)
                nc.scalar.dma_start(out=xt_sb[:, :, ms], in_=xt_src[:, :, ms])
            # Tail of x^T rides the SP ring: shortens the Act issue chain.
            nc.sync.dma_start(
                out=xt_sb[:, :, 2560:B], in_=xt_src[:, :, 2560:B]
            )

            # Batch chunks: 512-wide first (aligned with the k-split first
            # loads) and last (small final-store tail); 1024-pairs between.
            # Total matmul count stays at the floor of 256 (512 rows each).
            CHUNKS = [(0, 512), (512, 1536), (1536, 2560), (2560, 3584),
                      (3584, 4096)]
            # Store piece boundaries (in t-tiles) per chunk; rings rotate.
            PIECES = [0, 3, 6, 9, 12, 14, NT]
            RINGS = [nc.sync, nc.scalar, nc.gpsimd]

            out_r = out.rearrange("(t p) m -> p t m", p=P)
            for m, (mlo, mhi) in enumerate(CHUNKS):
                csz = mhi - mlo
                ms = slice(mlo, mhi)
                # Fixed shape so the pool ring-rotates 2 slots; small
                # chunks use a prefix slice.
                stage_full = stage_pool.tile([P, NT, M_PAIR], bf)
                stage = stage_full[:, :, 0:csz]
                for t in range(NT):
                    ns = slice(t * P, (t + 1) * P)
                    # Fixed 2-bank tile keeps PSUM bank alignment; small
                    # chunks use a prefix slice.
                    ps = psum_pool.tile([P, M_PAIR], f32)
                    for k in range(KO):
                        st, sp = (k == 0), (k == KO - 1)
                        # One LDWEIGHTS per (k, t) covers every 512-wide
                        # moving slice of this chunk.
                        for s0 in range(0, csz, M_TILE):
                            s1 = min(s0 + M_TILE, csz)
                            nc.tensor.matmul(
                                ps[:, s0:s1], lhsT=w_sb[:, k, ns],
                                rhs=xt_sb[:, k, mlo + s0:mlo + s1],
                                start=st, stop=sp,
                            )
                    # PSUM->SBUF eviction with fused bias add + bf16 cast,
                    # alternating DVE / Act (Pool cannot read PSUM on TRN2).
                    dst = stage[:, t, :]
                    bs = bias_sb[:, t:t + 1]
                    if t % 2 == 0:
                        nc.vector.tensor_scalar_add(dst, ps[:, 0:csz], bs)
                    else:
                        nc.scalar.activation(
                            dst, ps[:, 0:csz],
                            mybir.ActivationFunctionType.Identity,
                            bias=bs, scale=1.0,
                        )
                # Stores: ~6 pieces per chunk rotated over the three rings
                # (SP + Act HWDGE, Pool SWDGE) -- each fires as soon as its
                # tiles evict, keeping the wire evenly fed.  The final
                # chunk goes extra-fine so the very last transfer is tiny.
                pieces = (list(range(0, NT + 1, 2))
                          if m == len(CHUNKS) - 1 else PIECES)
                for i in range(len(pieces) - 1):
                    lo, hi = pieces[i], pieces[i + 1]
                    eng = RINGS[(m + i) % 3]
                    eng.dma_start(
                        out=out_r[:, lo:hi, ms], in_=stage[:, lo:hi, :]
                    )

    nc.compile()
    return nc


def _get_program(mode=None):
    if "prog" not in _CACHE:
        _CACHE["prog"] = _build_program()
    return _CACHE["prog"]


def _shard_inputs(x, weights, bias, mode=None):
    # Fold the constant block-diagonal mask into the weights on the host.
    col_block = np.arange(IO, dtype=np.int64) // OUT_SIZE
    mask = (col_block[None, :] != np.arange(IN_SIZE)[:, None])
    wm = weights * mask.astype(weights.dtype)
    xt16 = x.T.astype(BF16).reshape(KO, P, B)
    in_maps = []
    for c in range(N_CORES):
        sl = slice(c * N_SHARD, (c + 1) * N_SHARD)
        w16 = wm[:, sl].astype(BF16).reshape(KO, P, N_SHARD)
        bias_t = np.ascontiguousarray(
            bias[sl].astype(np.float32).reshape(NT, P).T
        )
        in_maps.append({
            "xt": xt16,
            "w": np.ascontiguousarray(w16),
            "bias": bias_t,
        })
    return in_maps


def run_sharded(in_maps, mode=None, **kwargs):
    """Run the SPMD program on cores 0-7. kwargs forwarded (e.g. trace)."""
    from concourse.bass_utils import run_bass_kernel_spmd

    nc = _get_program()
    return run_bass_kernel_spmd(
        nc, in_maps, core_ids=list(range(N_CORES)), **kwargs
    )


def kernel(x: np.ndarray, weights: np.ndarray, bias: np.ndarray) -> np.ndarray:
    x = np.asarray(x, dtype=np.float32)
    weights = np.asarray(weights, dtype=np.float32)
    bias = np.asarray(bias, dtype=np.float32)
    in_maps = _shard_inputs(x, weights, bias)
    res = run_sharded(in_maps)
    # Each core returns out^T [N_SHARD, B] bf16; transpose back and upcast.
    full = np.concatenate(
        [np.asarray(res.results[c]["out"]).T for c in range(N_CORES)], axis=1
    ).astype(np.float32)
    return full.reshape(B, IN_SIZE, OUT_SIZE)
